# revision 8
# baseline (speedup 1.0000x reference)
"""Distributed 2-layer GCN on 8 TRN2 NeuronCores (Bass/Tile), v2.

Reference computation (PyG-style GCNConv, f32):
    e  = embed_table[node_tokens]            # [N, 256]
    x0 = e @ Wn^T + bn                       # [N, 128]
    h1 = Ahat @ (x0 @ w1^T) + b1 ; z1 = relu(h1)
    h2 = Ahat @ (z1 @ w2^T) + b2             # output [N, 128]
  with Ahat = D^-1/2 (A + I) D^-1/2, deg from dst(+self loops).
  (Ahat x) @ w^T == Ahat (x @ w^T), so we aggregate first and project after.

Sharding: nodes are partitioned contiguously across the 8 cores (6250 each,
padded to 6272 = 49 tiles of 128). Each core computes x0 for its own nodes,
all-gathers the full feature matrix between layers, aggregates the edges
pointing at its own nodes, projects, and writes its output shard.

v2 design (changes vs v1 in [brackets]):
  - [table2] x0 is a pure embedding gather: host precomputes
    table2 = embed_table @ Wn^T + bn (f32, cast bf16) so the whole input
    stage is a 256B-row dma_gather (lo/hi split for int16 indices; each
    slot fetches both halves, one of which is a zero row, and DVE adds).
    No on-chip projection pipeline at all.
  - Features between layers are bf16 [*, 128] (256B rows); PSUM accum f32.
  - [A/B split] Each core's shard is split at row 3200 (25 tiles / 24
    tiles). AllGather runs as TWO collectives (A rows -> z_fullA [25600],
    B rows -> z_fullB [24576]); both outputs are int16-indexable, which
    replaces v1's lo/hi halves AND lets each collective start as soon as
    its rows are ready / overlap with compute on the other block.
  - Edges (+self loops) are bucketed per (dst tile, src A/B view); chunk
    counts are maxed over the 8 cores so all cores run one SPMD program.
    No dedup (R stays strictly one-hot per slot).
  - [on-chip R] Host ships only per-slot metadata (dst column f32, norm
    f32, ~0.6MB). The [128 slot x 128 dst] R matrix of each chunk is
    built by one DVE op: R = (iota == dstcol) * norm, replacing v1's
    38.6MB of streamed rmeta. Self-loop R_t = identity * dinv^2[tile t]
    is built the same way.
  - [SBUF shards] The core's own z0/z1 shard stays resident in SBUF
    ([128, 49, 128] bf16), so self-loop contributions need no DMA.
  - [LAG pipeline] Per layer, blocks are processed A0..A3, B0, A4, B1 ...
    (LAG=3): B-view chunks of group g run after A-view chunks of group
    g+3, so the B collective has ~4 blocks of compute to hide under.
    Group PSUMs are [128, 4, 128] f32 (one full bank); <=4 live at once.
  - dma_gather fetches up to 8 chunks (1024 rows) per instruction,
    rotating over the 4 SWDGE queues.
  - Per dst tile close: aggT -> (scalar copy) -> w^T matmul -> bias(+relu)
    -> TensorE transpose -> z1_sb / staging -> DMA out.
"""

import os

import numpy as np

import concourse.bacc as bacc
import concourse.mybir as mybir
import concourse.tile as tile
from concourse.bass_utils import run_bass_kernel_spmd
from concourse.library_config import mlp

# Problem shape (hardcoded per harness contract)
N = 50000
E = 600000
V = 50000
D_IN = 256
D = 128
NCORES = 8

NPC = N // NCORES            # 6250 nodes per core
TPC = (NPC + 127) // 128     # 49 tiles per core
NPAD = TPC * 128             # 6272 padded nodes per core
ATILES = 25                  # tiles in the A block
AR = ATILES * 128            # 3200 A rows per core
BR = NPAD - AR               # 3072 B rows per core
NA = NCORES * AR             # 25600 rows in z_fullA (int16-indexable)
NB = NCORES * BR             # 24576 rows in z_fullB
VLO = V // 2                 # 25000: embedding-table split
EGT = 7                      # tiles per embedding gather group
ENG = TPC // EGT             # embedding groups
GRP = 4                      # dst tiles per aggregation group (PSUM bank)
LAG = 3                      # A-blocks processed ahead of B-blocks
GMAXC = 8                    # max chunks (x128 slots) per dma_gather
NQ = 4                       # SWDGE queues
F32 = mybir.dt.float32
BF16 = mybir.dt.bfloat16
I16 = mybir.dt.int16
STAGE = int(os.environ.get("KSTAGE", "4"))


def _wrap_idx(idx_linear):
    """[n] -> [128, n/16] int16: position j at [j%16, j//16], replicated x8."""
    n = idx_linear.shape[0]
    assert n % 16 == 0
    w = idx_linear.astype(np.int16).reshape(-1, 16).T
    return np.tile(w, (8, 1))


def _groups():
    gs = []
    t = 0
    while t < TPC:
        gs.append(list(range(t, min(t + GRP, TPC))))
        t += GRP
    return gs


def _block_seq(ng):
    """A/B block processing order: A0..A_LAG, then B_g after A_{g+LAG}."""
    seq = []
    for g in range(ng):
        seq.append((g, 0))
        if g >= LAG:
            seq.append((g - LAG, 1))
    for g in range(max(ng - LAG, 0), ng):
        seq.append((g, 1))
    return seq


def _preprocess(node_tokens, edge_index):
    """Build per-core host arrays + the (core-uniform) chunk schedule."""
    import ml_dtypes

    src = np.asarray(edge_index[0], dtype=np.int64)
    dst = np.asarray(edge_index[1], dtype=np.int64)
    tok = np.asarray(node_tokens, dtype=np.int64)

    deg = (np.bincount(dst, minlength=N) + 1).astype(np.float32)
    dinv = (1.0 / np.sqrt(deg)).astype(np.float32)

    core = dst // NPC
    dloc = dst % NPC
    tloc = dloc // 128
    dcol = (dloc % 128).astype(np.float32)
    norm = (dinv[src] * dinv[dst]).astype(np.float32)
    srcc = src // NPC
    srcr = src % NPC
    ab = (srcr >= AR).astype(np.int64)          # 0 = A view, 1 = B view
    idx16 = np.where(ab == 0, srcc * AR + srcr, srcc * BR + (srcr - AR))

    key = (core * TPC + tloc) * 2 + ab
    order = np.argsort(key, kind="stable")
    idx16_s = idx16[order]
    dcol_s = dcol[order]
    norm_s = norm[order]
    counts = np.bincount(key[order], minlength=NCORES * TPC * 2).reshape(
        NCORES, TPC, 2)
    starts = np.zeros(NCORES * TPC * 2 + 1, dtype=np.int64)
    np.cumsum(counts.reshape(-1), out=starts[1:])

    # chunks per (tile, view): max over cores
    cnt = np.maximum(1, -(-counts.max(axis=0) // 128))  # [TPC, 2]

    # linear chunk order: LAG-interleaved A/B blocks per group.
    # sched: per chunk (tile, first-of-tile, last-of-tile)
    groups = _groups()
    sched = []
    chunk_view = []
    chunk_of = {}       # (t, v) -> first linear chunk index
    for g, v in _block_seq(len(groups)):
        for t in groups[g]:
            chunk_of[(t, v)] = len(sched)
            k = int(cnt[t, v])
            for i in range(k):
                sched.append((t, v == 0 and i == 0, v == 1 and i == k - 1))
                chunk_view.append(v)
    tot_chunks = len(sched)

    # gathers: greedy runs of same-view consecutive chunks, up to GMAXC
    gathers = []        # (chunk_off, n_chunks, view)
    i = 0
    while i < tot_chunks:
        v = chunk_view[i]
        j = i
        while j < tot_chunks and j - i < GMAXC and chunk_view[j] == v:
            j += 1
        gathers.append((i, j - i, v))
        i = j

    per_core = []
    for c in range(NCORES):
        idx_lin = np.zeros(tot_chunks * 128, np.int64)
        dstc = np.zeros((128, tot_chunks), np.float32)
        nrm = np.zeros((128, tot_chunks), np.float32)
        for t in range(TPC):
            for v in (0, 1):
                k = (c * TPC + t) * 2 + v
                s0, ne = starts[k], int(counts[c, t, v])
                base = chunk_of[(t, v)] * 128
                sl = np.arange(base, base + ne)
                idx_lin[sl] = idx16_s[s0 : s0 + ne]
                dstc[sl % 128, sl // 128] = dcol_s[s0 : s0 + ne]
                nrm[sl % 128, sl // 128] = norm_s[s0 : s0 + ne]

        blocks = []
        for off, n, _v in gathers:
            blocks.append(_wrap_idx(idx_lin[off * 128 : (off + n) * 128]))
        gidx = np.concatenate(blocks, axis=1)

        # embedding gather indices (per 7-tile group, lo then hi)
        tc_ = tok[c * NPC : (c + 1) * NPC]
        tpad = np.concatenate([tc_, np.zeros(NPAD - NPC, np.int64)])
        lo = np.where(tpad < VLO, tpad, VLO)       # VLO = appended zero row
        hi = np.where(tpad >= VLO, tpad - VLO, V - VLO)
        eblocks = []
        for g in range(ENG):
            sl = slice(g * EGT * 128, (g + 1) * EGT * 128)
            eblocks.append(_wrap_idx(lo[sl]))
            eblocks.append(_wrap_idx(hi[sl]))
        eidx = np.concatenate(eblocks, axis=1)

        # self-loop scale dinv^2 per node, laid out [128, TPC]
        dv2 = np.zeros((128, TPC), np.float32)
        nodes = np.arange(NPC)
        dv2[nodes % 128, nodes // 128] = dinv[c * NPC : (c + 1) * NPC] ** 2

        per_core.append({"gidx": gidx, "eidx": eidx, "dstc": dstc,
                         "nrm": nrm, "dv2": dv2})

    layout = {"sched": sched, "gathers": gathers, "tot_chunks": tot_chunks}
    return per_core, layout


def _build(layout):
    sched = layout["sched"]
    gathers = layout["gathers"]
    tot_chunks = layout["tot_chunks"]
    GCOLS = tot_chunks * 8
    ECOLS = ENG * 2 * EGT * 8

    nc = bacc.Bacc("TRN2", target_bir_lowering=False, debug=False,
                   num_devices=NCORES, num_swdge_queues=NQ)

    tab_lo = nc.dram_tensor("tab_lo", [VLO + 1, D], BF16, kind="ExternalInput")
    tab_hi = nc.dram_tensor("tab_hi", [V - VLO + 1, D], BF16,
                            kind="ExternalInput")
    eidx_d = nc.dram_tensor("eidx", [128, ECOLS], I16, kind="ExternalInput")
    gidx_d = nc.dram_tensor("gidx", [128, GCOLS], I16, kind="ExternalInput")
    dstc_d = nc.dram_tensor("dstc", [128, tot_chunks], F32,
                            kind="ExternalInput")
    nrm_d = nc.dram_tensor("nrm", [128, tot_chunks], F32,
                           kind="ExternalInput")
    dv2_d = nc.dram_tensor("dv2", [128, TPC], F32, kind="ExternalInput")
    w1t_d = nc.dram_tensor("w1t", [128, D], BF16, kind="ExternalInput")
    w2t_d = nc.dram_tensor("w2t", [128, D], BF16, kind="ExternalInput")
    bias_d = nc.dram_tensor("bias", [128, 2], F32, kind="ExternalInput")
    iota_d = nc.dram_tensor("iota", [128, 128], BF16, kind="ExternalInput")
    identb_d = nc.dram_tensor("identb", [128, 128], BF16, kind="ExternalInput")
    out_d = nc.dram_tensor("out", [NPAD, D], F32, kind="ExternalOutput")

    ACT = mybir.ActivationFunctionType
    ALU = mybir.AluOpType

    with tile.TileContext(nc) as tc:
        with (
            tc.tile_pool(name="const", bufs=1) as cp,
            tc.tile_pool(name="embg", bufs=2) as embg,
            tc.tile_pool(name="msgs", bufs=8) as msgp,
            tc.tile_pool(name="rmat", bufs=8) as rmp,
            tc.tile_pool(name="work", bufs=3) as wk,
            tc.tile_pool(name="stage", bufs=2) as stg,
            tc.tile_pool(name="psA", bufs=5, space="PSUM") as psA,
            tc.tile_pool(name="psB", bufs=2, space="PSUM") as psB,
            tc.tile_pool(name="psC", bufs=1, space="PSUM") as psC,
            tc.tile_pool(name="dram", bufs=1, space="DRAM") as dram,
        ):
            nc.gpsimd.load_library(mlp)

            eidx_sb = cp.tile([128, ECOLS], I16)
            gidx_sb = cp.tile([128, GCOLS], I16)
            dstc_sb = cp.tile([128, tot_chunks], F32)
            nrm_sb = cp.tile([128, tot_chunks], F32)
            dv2_sb = cp.tile([128, TPC], F32)
            w1t_sb = cp.tile([128, D], BF16)
            w2t_sb = cp.tile([128, D], BF16)
            bias_sb = cp.tile([128, 2], F32)
            iota_sb = cp.tile([128, 128], BF16)
            identb_sb = cp.tile([128, 128], BF16)
            z0_sb = cp.tile([128, TPC, D], BF16)
            z1_sb = cp.tile([128, TPC, D], BF16)
            nc.sync.dma_start(eidx_sb[:], eidx_d[:])
            nc.sync.dma_start(gidx_sb[:], gidx_d[:])
            nc.sync.dma_start(dstc_sb[:], dstc_d[:])
            nc.sync.dma_start(nrm_sb[:], nrm_d[:])
            nc.sync.dma_start(dv2_sb[:], dv2_d[:])
            nc.sync.dma_start(w1t_sb[:], w1t_d[:])
            nc.sync.dma_start(w2t_sb[:], w2t_d[:])
            nc.sync.dma_start(bias_sb[:], bias_d[:])
            nc.sync.dma_start(iota_sb[:], iota_d[:])
            nc.sync.dma_start(identb_sb[:], identb_d[:])

            z0A = dram.tile([AR, D], BF16)
            z0B = dram.tile([BR, D], BF16)
            z1A = dram.tile([AR, D], BF16)
            z1B = dram.tile([BR, D], BF16)
            z0_fullA = dram.tile([NA, D], BF16, addr_space="Shared")
            z0_fullB = dram.tile([NB, D], BF16, addr_space="Shared")
            z1_fullA = dram.tile([NA, D], BF16, addr_space="Shared")
            z1_fullB = dram.tile([NB, D], BF16, addr_space="Shared")

            qn = [0]

            def next_q():
                qn[0] = (qn[0] + 1) % NQ
                return qn[0]

            def allgather(loc, full):
                nc.gpsimd.collective_compute(
                    "AllGather", mybir.AluOpType.bypass,
                    replica_groups=[list(range(NCORES))],
                    ins=[loc.opt()], outs=[full.opt()])

            def flushA(z_sb, locA, fullA):
                nc.sync.dma_start(
                    locA[:].rearrange("(t p) f -> p t f", p=128),
                    z_sb[:, 0:ATILES, :])
                if STAGE >= 2:
                    allgather(locA, fullA)

            def flushB(z_sb, locB, fullB):
                nc.sync.dma_start(
                    locB[:].rearrange("(t p) f -> p t f", p=128),
                    z_sb[:, ATILES:TPC, :])
                if STAGE >= 2:
                    allgather(locB, fullB)

            # ---- embedding stage: z0 = table2[tok] (table2 = E@Wn^T+bn) ----
            for g in range(ENG):
                nidx = EGT * 128
                e_lo = embg.tile([128, EGT, D], BF16, name="e_lo", tag="e_lo")
                e_hi = embg.tile([128, EGT, D], BF16, name="e_hi", tag="e_hi")
                off = g * 2 * EGT * 8
                nc.gpsimd.dma_gather(e_lo[:], tab_lo[:],
                                     eidx_sb[:, off : off + EGT * 8],
                                     nidx, nidx, D, queue_num=next_q())
                nc.gpsimd.dma_gather(e_hi[:], tab_hi[:],
                                     eidx_sb[:, off + EGT * 8 : off + 2 * EGT * 8],
                                     nidx, nidx, D, queue_num=next_q())
                nc.vector.tensor_tensor(z0_sb[:, g * EGT : (g + 1) * EGT, :],
                                        e_lo[:], e_hi[:], ALU.add)
                if g == 3:      # tiles 0..27 done; A block = tiles 0..24
                    flushA(z0_sb, z0A, z0_fullA)
            flushB(z0_sb, z0B, z0_fullB)

            # ---- GCN layers ----
            def gcn_layer(fullA, fullB, zin_sb, wt_sb, bias_col, relu,
                          zout_sb, on_tile_done, dest, dest_f32):
                views = (fullA, fullB)
                groups = _groups()
                open_ps = {}        # group -> psum tile [128, GRP, 128]

                def open_tile(t):
                    g = t // GRP
                    first_in_bank = g not in open_ps
                    if first_in_bank:
                        open_ps[g] = psA.tile([128, GRP, 128], F32,
                                              name="agg", tag="pA")
                    rself = rmp.tile([128, 128], BF16, name="rs", tag="r")
                    nc.vector.tensor_scalar(
                        rself[:], identb_sb[:], dv2_sb[:, t : t + 1], None,
                        ALU.mult)
                    # start=True zeroes the whole 2KB zero region (one bank =
                    # one group of GRP tiles), so only the group's first
                    # matmul starts; siblings accumulate onto the zeroed bank.
                    nc.tensor.matmul(open_ps[g][:, t % GRP, :],
                                     zin_sb[:, t, :], rself[:],
                                     start=first_in_bank, stop=False)

                def close_group(g):
                    # runs after the bank's stop matmul: one bank-wide psum
                    # read (ordered after every matmul in the group), then
                    # per-tile projection.
                    tiles = groups[g]
                    ntile = len(tiles)
                    ps = open_ps.pop(g)
                    aggT4 = wk.tile([128, GRP, 128], BF16, name="aggT4",
                                    tag="aggT4")
                    nc.scalar.activation(aggT4[:, 0:ntile, :],
                                         ps[:, 0:ntile, :], ACT.Copy)
                    stg_t = None
                    if dest is not None:
                        stg_t = stg.tile([128, GRP, D],
                                         F32 if dest_f32 else BF16,
                                         name="stage1", tag="st1")
                    for i, t in enumerate(tiles):
                        yT_ps = psB.tile([128, 128], F32, name="yT", tag="pB")
                        nc.tensor.matmul(yT_ps[:], wt_sb[:], aggT4[:, i, :],
                                         start=True, stop=True)
                        yT_sb = wk.tile([128, 128], BF16, name="yT_sb",
                                        tag="yT_sb")
                        nc.scalar.activation(yT_sb[:], yT_ps[:],
                                             ACT.Relu if relu else ACT.Identity,
                                             bias=bias_col)
                        y_ps = psC.tile([128, 128], BF16, name="y", tag="pC")
                        nc.tensor.matmul(y_ps[:], yT_sb[:], identb_sb[:],
                                         is_transpose=True, start=True,
                                         stop=True)
                        if zout_sb is not None:
                            nc.scalar.activation(zout_sb[:, t, :], y_ps[:],
                                                 ACT.Copy)
                        else:
                            nc.scalar.activation(stg_t[:, i, :], y_ps[:],
                                                 ACT.Copy)
                    if dest is not None:
                        g0 = tiles[0]
                        dst_rows = dest[g0 * 128 : (g0 + ntile) * 128, :]
                        nc.sync.dma_start(
                            dst_rows.rearrange("(t p) f -> p t f", p=128),
                            stg_t[:, 0:ntile, :])
                    if on_tile_done is not None:
                        for t in tiles:
                            on_tile_done(t)

                for off, n, v in gathers:
                    msgs = msgp.tile([128, GMAXC, D], BF16, name="m", tag="m")
                    nc.gpsimd.dma_gather(
                        msgs[:, 0:n, :], views[v][:],
                        gidx_sb[:, off * 8 : (off + n) * 8],
                        n * 128, n * 128, D, queue_num=next_q())
                    for k in range(n):
                        t, first, last = sched[off + k]
                        if first:
                            open_tile(t)
                        ci = off + k
                        rmat = rmp.tile([128, 128], BF16, name="r", tag="r")
                        nc.vector.tensor_scalar(
                            rmat[:], iota_sb[:], dstc_sb[:, ci : ci + 1],
                            nrm_sb[:, ci : ci + 1], ALU.is_equal, ALU.mult)
                        bank_last = last and (t % GRP == GRP - 1
                                              or t == TPC - 1)
                        nc.tensor.matmul(open_ps[t // GRP][:, t % GRP, :],
                                         msgs[:, k, :], rmat[:],
                                         start=False, stop=bank_last)
                        if bank_last:
                            close_group(t // GRP)

            def l1_tile_done(t):
                if t == 27:     # tiles 0..24 (A rows) are all closed
                    flushA(z1_sb, z1A, z1_fullA)
                elif t == TPC - 1:
                    flushB(z1_sb, z1B, z1_fullB)

            if STAGE >= 3:
                gcn_layer(z0_fullA, z0_fullB, z0_sb, w1t_sb, bias_sb[:, 0:1],
                          True, z1_sb, l1_tile_done, None, False)
            if STAGE >= 4:
                gcn_layer(z1_fullA, z1_fullB, z1_sb, w2t_sb, bias_sb[:, 1:2],
                          False, None, None, out_d.ap(), True)

    nc.compile()
    return nc


_CACHE = {}


def _run(inputs, trace=False):
    import ml_dtypes

    node_tokens = np.asarray(inputs["node_tokens"])
    edge_index = np.asarray(inputs["edge_index"])
    embed_table = np.asarray(inputs["embed_table"], dtype=np.float32)
    Wn = np.asarray(inputs["W_node_w"], dtype=np.float32)
    bn = np.asarray(inputs["W_node_b"], dtype=np.float32)
    w1 = np.asarray(inputs["w1"], dtype=np.float32)
    b1 = np.asarray(inputs["b1"], dtype=np.float32)
    w2 = np.asarray(inputs["w2"], dtype=np.float32)
    b2 = np.asarray(inputs["b2"], dtype=np.float32)

    per_core, layout = _preprocess(node_tokens, edge_index)

    if "nc" not in _CACHE:
        _CACHE["nc"] = _build(layout)
    nc = _CACHE["nc"]

    table2 = embed_table @ Wn.T + bn            # [V, 128] f32
    tab_lo = np.concatenate([table2[:VLO], np.zeros((1, D), np.float32)]
                            ).astype(ml_dtypes.bfloat16)
    tab_hi = np.concatenate([table2[VLO:], np.zeros((1, D), np.float32)]
                            ).astype(ml_dtypes.bfloat16)
    bias = np.stack([b1, b2], axis=1).astype(np.float32)
    iota = np.tile(np.arange(128, dtype=np.float32), (128, 1)
                   ).astype(ml_dtypes.bfloat16)
    identb = np.eye(128, dtype=ml_dtypes.bfloat16)

    in_maps = []
    for c in range(NCORES):
        in_maps.append({
            "tab_lo": tab_lo, "tab_hi": tab_hi,
            "eidx": per_core[c]["eidx"],
            "gidx": per_core[c]["gidx"],
            "dstc": per_core[c]["dstc"],
            "nrm": per_core[c]["nrm"],
            "dv2": per_core[c]["dv2"],
            "w1t": w1.T.astype(ml_dtypes.bfloat16),
            "w2t": w2.T.astype(ml_dtypes.bfloat16),
            "bias": bias, "iota": iota, "identb": identb,
        })

    res = run_bass_kernel_spmd(nc, in_maps, core_ids=list(range(NCORES)),
                               trace=trace)
    out = np.concatenate([res.results[c]["out"][:NPC] for c in range(NCORES)],
                         axis=0)
    return out.astype(np.float32), res


def kernel(**inputs):
    out, _ = _run(inputs, trace=False)
    return out


# revision 14
# speedup vs baseline: 1.2438x; 1.2438x over previous
"""Distributed 2-layer GCN on 8 TRN2 NeuronCores (Bass/Tile), v2.

Reference computation (PyG-style GCNConv, f32):
    e  = embed_table[node_tokens]            # [N, 256]
    x0 = e @ Wn^T + bn                       # [N, 128]
    h1 = Ahat @ (x0 @ w1^T) + b1 ; z1 = relu(h1)
    h2 = Ahat @ (z1 @ w2^T) + b2             # output [N, 128]
  with Ahat = D^-1/2 (A + I) D^-1/2, deg from dst(+self loops).
  (Ahat x) @ w^T == Ahat (x @ w^T), so we aggregate first and project after.

Sharding: nodes are partitioned contiguously across the 8 cores (6250 each,
padded to 6272 = 49 tiles of 128). Each core computes x0 for its own nodes,
all-gathers the full feature matrix between layers, aggregates the edges
pointing at its own nodes, projects, and writes its output shard.

v2 design (changes vs v1 in [brackets]):
  - [table2] x0 is a pure embedding gather: host precomputes
    table2 = embed_table @ Wn^T + bn (f32, cast bf16) so the whole input
    stage is a 256B-row dma_gather (lo/hi split for int16 indices; each
    slot fetches both halves, one of which is a zero row, and DVE adds).
    No on-chip projection pipeline at all.
  - Features between layers are bf16 [*, 128] (256B rows); PSUM accum f32.
  - [A/B split] Each core's shard is split at row 3200 (25 tiles / 24
    tiles). AllGather runs as TWO collectives (A rows -> z_fullA [25600],
    B rows -> z_fullB [24576]); both outputs are int16-indexable, which
    replaces v1's lo/hi halves AND lets each collective start as soon as
    its rows are ready / overlap with compute on the other block.
  - Edges (+self loops) are bucketed per (dst tile, src A/B view); chunk
    counts are maxed over the 8 cores so all cores run one SPMD program.
    No dedup (R stays strictly one-hot per slot).
  - [on-chip R] Host ships only per-slot metadata (dst column f32, norm
    f32, ~0.6MB). The [128 slot x 128 dst] R matrix of each chunk is
    built by one DVE op: R = (iota == dstcol) * norm, replacing v1's
    38.6MB of streamed rmeta. Self-loop R_t = identity * dinv^2[tile t]
    is built the same way.
  - [SBUF shards] The core's own z0/z1 shard stays resident in SBUF
    ([128, 49, 128] bf16), so self-loop contributions need no DMA.
  - [LAG pipeline] Per layer, blocks are processed A0..A3, B0, A4, B1 ...
    (LAG=3): B-view chunks of group g run after A-view chunks of group
    g+3, so the B collective has ~4 blocks of compute to hide under.
    Group PSUMs are [128, 4, 128] f32 (one full bank); <=4 live at once.
  - dma_gather fetches up to 8 chunks (1024 rows) per instruction,
    rotating over the 4 SWDGE queues.
  - Per dst tile close: aggT -> (scalar copy) -> w^T matmul -> bias(+relu)
    -> TensorE transpose -> z1_sb / staging -> DMA out.
"""

import os

import numpy as np

import concourse.bacc as bacc
import concourse.mybir as mybir
import concourse.tile as tile
from concourse.bass_utils import run_bass_kernel_spmd
from concourse.library_config import mlp

# Problem shape (hardcoded per harness contract)
N = 50000
E = 600000
V = 50000
D_IN = 256
D = 128
NCORES = 8

NPC = N // NCORES            # 6250 nodes per core
TPC = (NPC + 127) // 128     # 49 tiles per core
NPAD = TPC * 128             # 6272 padded nodes per core
ATILES = 25                  # tiles in the A block
AR = ATILES * 128            # 3200 A rows per core
BR = NPAD - AR               # 3072 B rows per core
NA = NCORES * AR             # 25600 rows in z_fullA (int16-indexable)
NB = NCORES * BR             # 24576 rows in z_fullB
VLO = V // 2                 # 25000: embedding-table split
EGT = 7                      # tiles per embedding gather group
ENG = TPC // EGT             # embedding groups
GRP = 4                      # dst tiles per aggregation group (PSUM bank)
LAG = 3                      # A-blocks processed ahead of B-blocks
GMAXC = 8                    # max chunks (x128 slots) per dma_gather
NQ = 4                       # SWDGE queues
F32 = mybir.dt.float32
BF16 = mybir.dt.bfloat16
I16 = mybir.dt.int16
STAGE = int(os.environ.get("KSTAGE", "4"))


def _wrap_idx(idx_linear):
    """[n] -> [128, n/16] int16: position j at [j%16, j//16], replicated x8."""
    n = idx_linear.shape[0]
    assert n % 16 == 0
    w = idx_linear.astype(np.int16).reshape(-1, 16).T
    return np.tile(w, (8, 1))


def _groups():
    gs = []
    t = 0
    while t < TPC:
        gs.append(list(range(t, min(t + GRP, TPC))))
        t += GRP
    return gs


def _block_seq(ng):
    """A/B block processing order: A0..A_LAG, then B_g after A_{g+LAG}."""
    seq = []
    for g in range(ng):
        seq.append((g, 0))
        if g >= LAG:
            seq.append((g - LAG, 1))
    for g in range(max(ng - LAG, 0), ng):
        seq.append((g, 1))
    return seq


def _preprocess(node_tokens, edge_index):
    """Build per-core host arrays + the (core-uniform) chunk schedule."""
    import ml_dtypes

    src = np.asarray(edge_index[0], dtype=np.int64)
    dst = np.asarray(edge_index[1], dtype=np.int64)
    tok = np.asarray(node_tokens, dtype=np.int64)

    deg = (np.bincount(dst, minlength=N) + 1).astype(np.float32)
    dinv = (1.0 / np.sqrt(deg)).astype(np.float32)

    core = dst // NPC
    dloc = dst % NPC
    tloc = dloc // 128
    dcol = (dloc % 128).astype(np.float32)
    norm = (dinv[src] * dinv[dst]).astype(np.float32)
    srcc = src // NPC
    srcr = src % NPC
    ab = (srcr >= AR).astype(np.int64)          # 0 = A view, 1 = B view
    idx16 = np.where(ab == 0, srcc * AR + srcr, srcc * BR + (srcr - AR))

    key = (core * TPC + tloc) * 2 + ab
    order = np.argsort(key, kind="stable")
    idx16_s = idx16[order]
    dcol_s = dcol[order]
    norm_s = norm[order]
    counts = np.bincount(key[order], minlength=NCORES * TPC * 2).reshape(
        NCORES, TPC, 2)
    starts = np.zeros(NCORES * TPC * 2 + 1, dtype=np.int64)
    np.cumsum(counts.reshape(-1), out=starts[1:])

    # chunks per (tile, view): max over cores
    cnt = np.maximum(1, -(-counts.max(axis=0) // 128))  # [TPC, 2]

    # linear chunk order: LAG-interleaved A/B blocks per group.
    # sched: per chunk (tile, first-of-tile, last-of-tile)
    groups = _groups()
    sched = []
    chunk_view = []
    chunk_of = {}       # (t, v) -> first linear chunk index
    for g, v in _block_seq(len(groups)):
        for t in groups[g]:
            chunk_of[(t, v)] = len(sched)
            k = int(cnt[t, v])
            for i in range(k):
                sched.append((t, v == 0 and i == 0, v == 1 and i == k - 1))
                chunk_view.append(v)
    tot_chunks = len(sched)

    # gathers: greedy runs of same-view consecutive chunks, up to GMAXC
    gathers = []        # (chunk_off, n_chunks, view)
    i = 0
    while i < tot_chunks:
        v = chunk_view[i]
        j = i
        while j < tot_chunks and j - i < GMAXC and chunk_view[j] == v:
            j += 1
        gathers.append((i, j - i, v))
        i = j

    per_core = []
    for c in range(NCORES):
        idx_lin = np.zeros(tot_chunks * 128, np.int64)
        dstc = np.zeros((128, tot_chunks), np.float32)
        nrm = np.zeros((128, tot_chunks), np.float32)
        for t in range(TPC):
            for v in (0, 1):
                k = (c * TPC + t) * 2 + v
                s0, ne = starts[k], int(counts[c, t, v])
                base = chunk_of[(t, v)] * 128
                sl = np.arange(base, base + ne)
                idx_lin[sl] = idx16_s[s0 : s0 + ne]
                dstc[sl % 128, sl // 128] = dcol_s[s0 : s0 + ne]
                nrm[sl % 128, sl // 128] = norm_s[s0 : s0 + ne]

        blocks = []
        for off, n, _v in gathers:
            blocks.append(_wrap_idx(idx_lin[off * 128 : (off + n) * 128]))
        gidx = np.concatenate(blocks, axis=1)

        # embedding gather indices (per 7-tile group, lo then hi)
        tc_ = tok[c * NPC : (c + 1) * NPC]
        tpad = np.concatenate([tc_, np.zeros(NPAD - NPC, np.int64)])
        lo = np.where(tpad < VLO, tpad, VLO)       # VLO = appended zero row
        hi = np.where(tpad >= VLO, tpad - VLO, V - VLO)
        eblocks = []
        for g in range(ENG):
            sl = slice(g * EGT * 128, (g + 1) * EGT * 128)
            eblocks.append(_wrap_idx(lo[sl]))
            eblocks.append(_wrap_idx(hi[sl]))
        eidx = np.concatenate(eblocks, axis=1)

        # self-loop scale dinv^2 per node, laid out [128, TPC]
        dv2 = np.zeros((128, TPC), np.float32)
        nodes = np.arange(NPC)
        dv2[nodes % 128, nodes // 128] = dinv[c * NPC : (c + 1) * NPC] ** 2

        per_core.append({"gidx": gidx, "eidx": eidx, "dstc": dstc,
                         "nrm": nrm, "dv2": dv2})

    layout = {"sched": sched, "gathers": gathers, "tot_chunks": tot_chunks}
    return per_core, layout


def _build(layout):
    sched = layout["sched"]
    gathers = layout["gathers"]
    tot_chunks = layout["tot_chunks"]
    GCOLS = tot_chunks * 8
    ECOLS = ENG * 2 * EGT * 8

    nc = bacc.Bacc("TRN2", target_bir_lowering=False, debug=False,
                   num_devices=NCORES, num_swdge_queues=NQ)

    tab_lo = nc.dram_tensor("tab_lo", [VLO + 1, D], BF16, kind="ExternalInput")
    tab_hi = nc.dram_tensor("tab_hi", [V - VLO + 1, D], BF16,
                            kind="ExternalInput")
    eidx_d = nc.dram_tensor("eidx", [128, ECOLS], I16, kind="ExternalInput")
    gidx_d = nc.dram_tensor("gidx", [128, GCOLS], I16, kind="ExternalInput")
    dstc_d = nc.dram_tensor("dstc", [128, tot_chunks], BF16,
                            kind="ExternalInput")
    nrm_d = nc.dram_tensor("nrm", [128, tot_chunks], BF16,
                           kind="ExternalInput")
    dv2_d = nc.dram_tensor("dv2", [128, TPC], BF16, kind="ExternalInput")
    w1t_d = nc.dram_tensor("w1t", [128, D], BF16, kind="ExternalInput")
    w2t_d = nc.dram_tensor("w2t", [128, D], BF16, kind="ExternalInput")
    bias_d = nc.dram_tensor("bias", [128, 2], F32, kind="ExternalInput")
    iota_d = nc.dram_tensor("iota", [128, 128], BF16, kind="ExternalInput")
    identb_d = nc.dram_tensor("identb", [128, 128], BF16, kind="ExternalInput")
    out_d = nc.dram_tensor("out", [NPAD, D], F32, kind="ExternalOutput")

    ACT = mybir.ActivationFunctionType
    ALU = mybir.AluOpType

    with tile.TileContext(nc) as tc:
        with (
            tc.tile_pool(name="const", bufs=1) as cp,
            tc.tile_pool(name="embg", bufs=4) as embg,
            tc.tile_pool(name="msgs", bufs=8) as msgp,
            tc.tile_pool(name="rmat", bufs=8) as rmp,
            tc.tile_pool(name="rself", bufs=3) as rsp,
            tc.tile_pool(name="work", bufs=3) as wk,
            tc.tile_pool(name="stage", bufs=2) as stg,
            tc.tile_pool(name="psA", bufs=5, space="PSUM") as psA,
            tc.tile_pool(name="psB", bufs=2, space="PSUM") as psB,
            tc.tile_pool(name="psC", bufs=1, space="PSUM") as psC,
            tc.tile_pool(name="dram", bufs=1, space="DRAM") as dram,
        ):
            nc.gpsimd.load_library(mlp)

            eidx_sb = cp.tile([128, ECOLS], I16)
            gidx_sb = cp.tile([128, GCOLS], I16)
            dstc_sb = cp.tile([128, tot_chunks], BF16)
            nrm_sb = cp.tile([128, tot_chunks], BF16)
            dv2_sb = cp.tile([128, TPC], BF16)
            w1t_sb = cp.tile([128, D], BF16)
            w2t_sb = cp.tile([128, D], BF16)
            bias_sb = cp.tile([128, 2], F32)
            iota_sb = cp.tile([128, 128], BF16)
            identb_sb = cp.tile([128, 128], BF16)
            z0_sb = cp.tile([128, TPC, D], BF16)
            z1_sb = cp.tile([128, TPC, D], BF16)
            nc.sync.dma_start(eidx_sb[:], eidx_d[:])
            nc.sync.dma_start(gidx_sb[:], gidx_d[:])
            nc.sync.dma_start(dstc_sb[:], dstc_d[:])
            nc.sync.dma_start(nrm_sb[:], nrm_d[:])
            nc.sync.dma_start(dv2_sb[:], dv2_d[:])
            nc.sync.dma_start(w1t_sb[:], w1t_d[:])
            nc.sync.dma_start(w2t_sb[:], w2t_d[:])
            nc.sync.dma_start(bias_sb[:], bias_d[:])
            nc.sync.dma_start(iota_sb[:], iota_d[:])
            nc.sync.dma_start(identb_sb[:], identb_d[:])

            z0A = dram.tile([AR, D], BF16)
            z0B = dram.tile([BR, D], BF16)
            z1A = dram.tile([AR, D], BF16)
            z1B = dram.tile([BR, D], BF16)
            z0_fullA = dram.tile([NA, D], BF16, addr_space="Shared")
            z0_fullB = dram.tile([NB, D], BF16, addr_space="Shared")
            z1_fullA = dram.tile([NA, D], BF16, addr_space="Shared")
            z1_fullB = dram.tile([NB, D], BF16, addr_space="Shared")

            qn = [0]

            def next_q():
                qn[0] = (qn[0] + 1) % NQ
                return qn[0]

            def allgather(loc, full):
                nc.gpsimd.collective_compute(
                    "AllGather", mybir.AluOpType.bypass,
                    replica_groups=[list(range(NCORES))],
                    ins=[loc.opt()], outs=[full.opt()])

            def flushA(z_sb, locA, fullA):
                nc.sync.dma_start(
                    locA[:].rearrange("(t p) f -> p t f", p=128),
                    z_sb[:, 0:ATILES, :])
                if STAGE >= 2:
                    allgather(locA, fullA)

            def flushB(z_sb, locB, fullB):
                nc.sync.dma_start(
                    locB[:].rearrange("(t p) f -> p t f", p=128),
                    z_sb[:, ATILES:TPC, :])
                if STAGE >= 2:
                    allgather(locB, fullB)

            # ---- embedding stage: z0 = table2[tok] (table2 = E@Wn^T+bn) ----
            for g in range(ENG):
                nidx = EGT * 128
                e_lo = embg.tile([128, EGT, D], BF16, name="e_lo", tag="e_lo")
                e_hi = embg.tile([128, EGT, D], BF16, name="e_hi", tag="e_hi")
                off = g * 2 * EGT * 8
                nc.gpsimd.dma_gather(e_lo[:], tab_lo[:],
                                     eidx_sb[:, off : off + EGT * 8],
                                     nidx, nidx, D, queue_num=next_q())
                nc.gpsimd.dma_gather(e_hi[:], tab_hi[:],
                                     eidx_sb[:, off + EGT * 8 : off + 2 * EGT * 8],
                                     nidx, nidx, D, queue_num=next_q())
                nc.vector.tensor_tensor(z0_sb[:, g * EGT : (g + 1) * EGT, :],
                                        e_lo[:], e_hi[:], ALU.add)
                if g == 3:      # tiles 0..27 done; A block = tiles 0..24
                    flushA(z0_sb, z0A, z0_fullA)
            flushB(z0_sb, z0B, z0_fullB)

            # ---- GCN layers ----
            def gcn_layer(fullA, fullB, zin_sb, wt_sb, bias_col, relu,
                          zout_sb, on_tile_done, dest, dest_f32):
                views = (fullA, fullB)
                groups = _groups()
                open_ps = {}        # group -> psum tile [128, GRP, 128]
                rself_t = {}        # group -> batched self-loop R

                def open_tile(t):
                    g = t // GRP
                    first_in_bank = g not in open_ps
                    if first_in_bank:
                        open_ps[g] = psA.tile([128, GRP, 128], F32,
                                              name="agg", tag="pA")
                        nt = len(groups[g])
                        rs = rsp.tile([128, GRP, 128], BF16, name="rs",
                                      tag="rs")
                        nc.vector.tensor_tensor(
                            rs[:, 0:nt, :],
                            identb_sb[:].unsqueeze(1)
                            .broadcast_to([128, nt, 128]),
                            dv2_sb[:, g * GRP : g * GRP + nt].unsqueeze(2)
                            .broadcast_to([128, nt, 128]),
                            ALU.mult)
                        rself_t[g] = rs
                    # start=True zeroes the whole 2KB zero region (one bank =
                    # one group of GRP tiles), so only the group's first
                    # matmul starts; siblings accumulate onto the zeroed bank.
                    nc.tensor.matmul(open_ps[g][:, t % GRP, :],
                                     zin_sb[:, t, :],
                                     rself_t[g][:, t % GRP, :],
                                     start=first_in_bank, stop=False)

                def close_group(g):
                    # runs after the bank's stop matmul: one bank-wide psum
                    # read (ordered after every matmul in the group), then
                    # per-tile projection.
                    tiles = groups[g]
                    ntile = len(tiles)
                    ps = open_ps.pop(g)
                    aggT4 = wk.tile([128, GRP, 128], BF16, name="aggT4",
                                    tag="aggT4")
                    nc.scalar.activation(aggT4[:, 0:ntile, :],
                                         ps[:, 0:ntile, :], ACT.Copy)
                    stg_t = None
                    if dest is not None:
                        stg_t = stg.tile([128, GRP, D],
                                         F32 if dest_f32 else BF16,
                                         name="stage1", tag="st1")
                    for i, t in enumerate(tiles):
                        yT_ps = psB.tile([128, 128], F32, name="yT", tag="pB")
                        nc.tensor.matmul(yT_ps[:], wt_sb[:], aggT4[:, i, :],
                                         start=True, stop=True)
                        yT_sb = wk.tile([128, 128], BF16, name="yT_sb",
                                        tag="yT_sb")
                        nc.scalar.activation(yT_sb[:], yT_ps[:],
                                             ACT.Relu if relu else ACT.Identity,
                                             bias=bias_col)
                        y_ps = psC.tile([128, 128], BF16, name="y", tag="pC")
                        nc.tensor.matmul(y_ps[:], yT_sb[:], identb_sb[:],
                                         is_transpose=True, start=True,
                                         stop=True)
                        if zout_sb is not None:
                            nc.scalar.activation(zout_sb[:, t, :], y_ps[:],
                                                 ACT.Copy)
                        else:
                            nc.scalar.activation(stg_t[:, i, :], y_ps[:],
                                                 ACT.Copy)
                    if dest is not None:
                        g0 = tiles[0]
                        dst_rows = dest[g0 * 128 : (g0 + ntile) * 128, :]
                        nc.sync.dma_start(
                            dst_rows.rearrange("(t p) f -> p t f", p=128),
                            stg_t[:, 0:ntile, :])
                    if on_tile_done is not None:
                        for t in tiles:
                            on_tile_done(t)

                for off, n, v in gathers:
                    msgs = msgp.tile([128, GMAXC, D], BF16, name="m", tag="m")
                    nc.gpsimd.dma_gather(
                        msgs[:, 0:n, :], views[v][:],
                        gidx_sb[:, off * 8 : (off + n) * 8],
                        n * 128, n * 128, D, queue_num=next_q())
                    # batched one-hot R for the gather's n chunks: two DVE
                    # ops instead of n tensor_scalars.
                    rb = rmp.tile([128, GMAXC, 128], BF16, name="r", tag="r")
                    nc.vector.tensor_tensor(
                        rb[:, 0:n, :],
                        iota_sb[:].unsqueeze(1).broadcast_to([128, n, 128]),
                        dstc_sb[:, off : off + n].unsqueeze(2)
                        .broadcast_to([128, n, 128]),
                        ALU.is_equal)
                    nc.vector.tensor_tensor(
                        rb[:, 0:n, :], rb[:, 0:n, :],
                        nrm_sb[:, off : off + n].unsqueeze(2)
                        .broadcast_to([128, n, 128]),
                        ALU.mult)
                    for k in range(n):
                        t, first, last = sched[off + k]
                        if first:
                            open_tile(t)
                        bank_last = last and (t % GRP == GRP - 1
                                              or t == TPC - 1)
                        nc.tensor.matmul(open_ps[t // GRP][:, t % GRP, :],
                                         msgs[:, k, :], rb[:, k, :],
                                         start=False, stop=bank_last)
                        if bank_last:
                            close_group(t // GRP)

            def l1_tile_done(t):
                if t == 27:     # tiles 0..24 (A rows) are all closed
                    flushA(z1_sb, z1A, z1_fullA)
                elif t == TPC - 1:
                    flushB(z1_sb, z1B, z1_fullB)

            if STAGE >= 3:
                gcn_layer(z0_fullA, z0_fullB, z0_sb, w1t_sb, bias_sb[:, 0:1],
                          True, z1_sb, l1_tile_done, None, False)
            if STAGE >= 4:
                gcn_layer(z1_fullA, z1_fullB, z1_sb, w2t_sb, bias_sb[:, 1:2],
                          False, None, None, out_d.ap(), True)

    nc.compile()
    return nc


_CACHE = {}


def _run(inputs, trace=False):
    import ml_dtypes

    node_tokens = np.asarray(inputs["node_tokens"])
    edge_index = np.asarray(inputs["edge_index"])
    embed_table = np.asarray(inputs["embed_table"], dtype=np.float32)
    Wn = np.asarray(inputs["W_node_w"], dtype=np.float32)
    bn = np.asarray(inputs["W_node_b"], dtype=np.float32)
    w1 = np.asarray(inputs["w1"], dtype=np.float32)
    b1 = np.asarray(inputs["b1"], dtype=np.float32)
    w2 = np.asarray(inputs["w2"], dtype=np.float32)
    b2 = np.asarray(inputs["b2"], dtype=np.float32)

    per_core, layout = _preprocess(node_tokens, edge_index)

    if "nc" not in _CACHE:
        _CACHE["nc"] = _build(layout)
    nc = _CACHE["nc"]

    table2 = embed_table @ Wn.T + bn            # [V, 128] f32
    tab_lo = np.concatenate([table2[:VLO], np.zeros((1, D), np.float32)]
                            ).astype(ml_dtypes.bfloat16)
    tab_hi = np.concatenate([table2[VLO:], np.zeros((1, D), np.float32)]
                            ).astype(ml_dtypes.bfloat16)
    bias = np.stack([b1, b2], axis=1).astype(np.float32)
    iota = np.tile(np.arange(128, dtype=np.float32), (128, 1)
                   ).astype(ml_dtypes.bfloat16)
    identb = np.eye(128, dtype=ml_dtypes.bfloat16)

    in_maps = []
    for c in range(NCORES):
        in_maps.append({
            "tab_lo": tab_lo, "tab_hi": tab_hi,
            "eidx": per_core[c]["eidx"],
            "gidx": per_core[c]["gidx"],
            "dstc": per_core[c]["dstc"].astype(ml_dtypes.bfloat16),
            "nrm": per_core[c]["nrm"].astype(ml_dtypes.bfloat16),
            "dv2": per_core[c]["dv2"].astype(ml_dtypes.bfloat16),
            "w1t": w1.T.astype(ml_dtypes.bfloat16),
            "w2t": w2.T.astype(ml_dtypes.bfloat16),
            "bias": bias, "iota": iota, "identb": identb,
        })

    res = run_bass_kernel_spmd(nc, in_maps, core_ids=list(range(NCORES)),
                               trace=trace)
    out = np.concatenate([res.results[c]["out"][:NPC] for c in range(NCORES)],
                         axis=0)
    return out.astype(np.float32), res


def kernel(**inputs):
    out, _ = _run(inputs, trace=False)
    return out


# revision 20
# speedup vs baseline: 1.2832x; 1.0317x over previous
"""Distributed 2-layer GCN on 8 TRN2 NeuronCores (Bass/Tile), v2.

Reference computation (PyG-style GCNConv, f32):
    e  = embed_table[node_tokens]            # [N, 256]
    x0 = e @ Wn^T + bn                       # [N, 128]
    h1 = Ahat @ (x0 @ w1^T) + b1 ; z1 = relu(h1)
    h2 = Ahat @ (z1 @ w2^T) + b2             # output [N, 128]
  with Ahat = D^-1/2 (A + I) D^-1/2, deg from dst(+self loops).
  (Ahat x) @ w^T == Ahat (x @ w^T), so we aggregate first and project after.

Sharding: nodes are partitioned contiguously across the 8 cores (6250 each,
padded to 6272 = 49 tiles of 128). Each core computes x0 for its own nodes,
all-gathers the full feature matrix between layers, aggregates the edges
pointing at its own nodes, projects, and writes its output shard.

v2 design (changes vs v1 in [brackets]):
  - [table2] x0 is a pure embedding gather: host precomputes
    table2 = embed_table @ Wn^T + bn (f32, cast bf16) so the whole input
    stage is a 256B-row dma_gather (lo/hi split for int16 indices; each
    slot fetches both halves, one of which is a zero row, and DVE adds).
    No on-chip projection pipeline at all.
  - Features between layers are bf16 [*, 128] (256B rows); PSUM accum f32.
  - [A/B split] Each core's shard is split at row 3200 (25 tiles / 24
    tiles). AllGather runs as TWO collectives (A rows -> z_fullA [25600],
    B rows -> z_fullB [24576]); both outputs are int16-indexable, which
    replaces v1's lo/hi halves AND lets each collective start as soon as
    its rows are ready / overlap with compute on the other block.
  - Edges (+self loops) are bucketed per (dst tile, src A/B view); chunk
    counts are maxed over the 8 cores so all cores run one SPMD program.
    No dedup (R stays strictly one-hot per slot).
  - [on-chip R] Host ships only per-slot metadata (dst column f32, norm
    f32, ~0.6MB). The [128 slot x 128 dst] R matrix of each chunk is
    built by one DVE op: R = (iota == dstcol) * norm, replacing v1's
    38.6MB of streamed rmeta. Self-loop R_t = identity * dinv^2[tile t]
    is built the same way.
  - [SBUF shards] The core's own z0/z1 shard stays resident in SBUF
    ([128, 49, 128] bf16), so self-loop contributions need no DMA.
  - [LAG pipeline] Per layer, blocks are processed A0..A3, B0, A4, B1 ...
    (LAG=3): B-view chunks of group g run after A-view chunks of group
    g+3, so the B collective has ~4 blocks of compute to hide under.
    Group PSUMs are [128, 4, 128] f32 (one full bank); <=4 live at once.
  - dma_gather fetches up to 8 chunks (1024 rows) per instruction,
    rotating over the 4 SWDGE queues.
  - Per dst tile close: aggT -> (scalar copy) -> w^T matmul -> bias(+relu)
    -> TensorE transpose -> z1_sb / staging -> DMA out.
"""

import os

import numpy as np

import concourse.bacc as bacc
import concourse.mybir as mybir
import concourse.tile as tile
from concourse.bass_utils import run_bass_kernel_spmd
from concourse.library_config import mlp

# Problem shape (hardcoded per harness contract)
N = 50000
E = 600000
V = 50000
D_IN = 256
D = 128
NCORES = 8

NPC = N // NCORES            # 6250 nodes per core
TPC = (NPC + 127) // 128     # 49 tiles per core
NPAD = TPC * 128             # 6272 padded nodes per core
ATILES = 21                  # tiles in the A block
AR = ATILES * 128            # 3200 A rows per core
BR = NPAD - AR               # 3072 B rows per core
NA = NCORES * AR             # 25600 rows in z_fullA (int16-indexable)
NB = NCORES * BR             # 24576 rows in z_fullB
VLO = V // 2                 # 25000: embedding-table split
EGT = 7                      # tiles per embedding gather group
ENG = TPC // EGT             # embedding groups
GRP = 4                      # dst tiles per aggregation group (PSUM bank)
LAG = 3                      # A-blocks processed ahead of B-blocks
GMAXC = 8                    # max chunks (x128 slots) per dma_gather
NQ = 4                       # SWDGE queues
F32 = mybir.dt.float32
BF16 = mybir.dt.bfloat16
I16 = mybir.dt.int16
STAGE = int(os.environ.get("KSTAGE", "4"))


def _wrap_idx(idx_linear):
    """[n] -> [128, n/16] int16: position j at [j%16, j//16], replicated x8."""
    n = idx_linear.shape[0]
    assert n % 16 == 0
    w = idx_linear.astype(np.int16).reshape(-1, 16).T
    return np.tile(w, (8, 1))


def _groups():
    gs = []
    t = 0
    while t < TPC:
        gs.append(list(range(t, min(t + GRP, TPC))))
        t += GRP
    return gs


def _block_seq(ng):
    """A/B block processing order: A0..A_LAG, then B_g after A_{g+LAG}."""
    seq = []
    for g in range(ng):
        seq.append((g, 0))
        if g >= LAG:
            seq.append((g - LAG, 1))
    for g in range(max(ng - LAG, 0), ng):
        seq.append((g, 1))
    return seq


def _preprocess(node_tokens, edge_index):
    """Build per-core host arrays + the (core-uniform) chunk schedule."""
    import ml_dtypes

    src = np.asarray(edge_index[0], dtype=np.int64)
    dst = np.asarray(edge_index[1], dtype=np.int64)
    tok = np.asarray(node_tokens, dtype=np.int64)

    deg = (np.bincount(dst, minlength=N) + 1).astype(np.float32)
    dinv = (1.0 / np.sqrt(deg)).astype(np.float32)

    core = dst // NPC
    dloc = dst % NPC
    tloc = dloc // 128
    dcol = (dloc % 128).astype(np.float32)
    norm = (dinv[src] * dinv[dst]).astype(np.float32)
    srcc = src // NPC
    srcr = src % NPC
    ab = (srcr >= AR).astype(np.int64)          # 0 = A view, 1 = B view
    idx16 = np.where(ab == 0, srcc * AR + srcr, srcc * BR + (srcr - AR))

    key = (core * TPC + tloc) * 2 + ab
    order = np.argsort(key, kind="stable")
    idx16_s = idx16[order]
    dcol_s = dcol[order]
    norm_s = norm[order]
    counts = np.bincount(key[order], minlength=NCORES * TPC * 2).reshape(
        NCORES, TPC, 2)
    starts = np.zeros(NCORES * TPC * 2 + 1, dtype=np.int64)
    np.cumsum(counts.reshape(-1), out=starts[1:])

    # chunks per (tile, view): max over cores
    cnt = np.maximum(1, -(-counts.max(axis=0) // 128))  # [TPC, 2]

    # linear chunk order: LAG-interleaved A/B blocks per group.
    # sched: per chunk (tile, first-of-tile, last-of-tile)
    groups = _groups()
    sched = []
    chunk_view = []
    chunk_of = {}       # (t, v) -> first linear chunk index
    for g, v in _block_seq(len(groups)):
        for t in groups[g]:
            chunk_of[(t, v)] = len(sched)
            k = int(cnt[t, v])
            for i in range(k):
                sched.append((t, v == 0 and i == 0, v == 1 and i == k - 1))
                chunk_view.append(v)
    tot_chunks = len(sched)

    # gathers: greedy runs of same-view consecutive chunks, up to GMAXC
    gathers = []        # (chunk_off, n_chunks, view)
    i = 0
    while i < tot_chunks:
        v = chunk_view[i]
        j = i
        while j < tot_chunks and j - i < GMAXC and chunk_view[j] == v:
            j += 1
        gathers.append((i, j - i, v))
        i = j

    per_core = []
    for c in range(NCORES):
        idx_lin = np.zeros(tot_chunks * 128, np.int64)
        dstc = np.zeros((128, tot_chunks), np.float32)
        nrm = np.zeros((128, tot_chunks), np.float32)
        for t in range(TPC):
            for v in (0, 1):
                k = (c * TPC + t) * 2 + v
                s0, ne = starts[k], int(counts[c, t, v])
                base = chunk_of[(t, v)] * 128
                sl = np.arange(base, base + ne)
                idx_lin[sl] = idx16_s[s0 : s0 + ne]
                dstc[sl % 128, sl // 128] = dcol_s[s0 : s0 + ne]
                nrm[sl % 128, sl // 128] = norm_s[s0 : s0 + ne]

        blocks = []
        for off, n, _v in gathers:
            blocks.append(_wrap_idx(idx_lin[off * 128 : (off + n) * 128]))
        gidx = np.concatenate(blocks, axis=1)

        # embedding gather indices (per 7-tile group, lo then hi)
        tc_ = tok[c * NPC : (c + 1) * NPC]
        tpad = np.concatenate([tc_, np.zeros(NPAD - NPC, np.int64)])
        lo = np.where(tpad < VLO, tpad, VLO)       # VLO = appended zero row
        hi = np.where(tpad >= VLO, tpad - VLO, V - VLO)
        eblocks = []
        for g in range(ENG):
            sl = slice(g * EGT * 128, (g + 1) * EGT * 128)
            eblocks.append(_wrap_idx(lo[sl]))
            eblocks.append(_wrap_idx(hi[sl]))
        eidx = np.concatenate(eblocks, axis=1)

        # self-loop scale dinv^2 per node, laid out [128, TPC]
        dv2 = np.zeros((128, TPC), np.float32)
        nodes = np.arange(NPC)
        dv2[nodes % 128, nodes // 128] = dinv[c * NPC : (c + 1) * NPC] ** 2

        per_core.append({"gidx": gidx, "eidx": eidx, "dstc": dstc,
                         "nrm": nrm, "dv2": dv2})

    layout = {"sched": sched, "gathers": gathers, "tot_chunks": tot_chunks}
    return per_core, layout


def _build(layout):
    sched = layout["sched"]
    gathers = layout["gathers"]
    tot_chunks = layout["tot_chunks"]
    GCOLS = tot_chunks * 8
    ECOLS = ENG * 2 * EGT * 8

    nc = bacc.Bacc("TRN2", target_bir_lowering=False, debug=False,
                   num_devices=NCORES, num_swdge_queues=NQ)

    tab_lo = nc.dram_tensor("tab_lo", [VLO + 1, D], BF16, kind="ExternalInput")
    tab_hi = nc.dram_tensor("tab_hi", [V - VLO + 1, D], BF16,
                            kind="ExternalInput")
    eidx_d = nc.dram_tensor("eidx", [128, ECOLS], I16, kind="ExternalInput")
    gidx_d = nc.dram_tensor("gidx", [128, GCOLS], I16, kind="ExternalInput")
    dstc_d = nc.dram_tensor("dstc", [128, tot_chunks], BF16,
                            kind="ExternalInput")
    nrm_d = nc.dram_tensor("nrm", [128, tot_chunks], BF16,
                           kind="ExternalInput")
    dv2_d = nc.dram_tensor("dv2", [128, TPC], BF16, kind="ExternalInput")
    w1t_d = nc.dram_tensor("w1t", [128, D], BF16, kind="ExternalInput")
    w2t_d = nc.dram_tensor("w2t", [128, D], BF16, kind="ExternalInput")
    bias_d = nc.dram_tensor("bias", [128, 2], F32, kind="ExternalInput")
    iota_d = nc.dram_tensor("iota", [128, 128], BF16, kind="ExternalInput")
    identb_d = nc.dram_tensor("identb", [128, 128], BF16, kind="ExternalInput")
    out_d = nc.dram_tensor("out", [NPAD, D], F32, kind="ExternalOutput")

    ACT = mybir.ActivationFunctionType
    ALU = mybir.AluOpType

    with tile.TileContext(nc) as tc:
        with (
            tc.tile_pool(name="const", bufs=1) as cp,
            tc.tile_pool(name="embg", bufs=4) as embg,
            tc.tile_pool(name="msgs", bufs=8) as msgp,
            tc.tile_pool(name="rmat", bufs=8) as rmp,
            tc.tile_pool(name="rself", bufs=3) as rsp,
            tc.tile_pool(name="work", bufs=3) as wk,
            tc.tile_pool(name="stage", bufs=2) as stg,
            tc.tile_pool(name="psA", bufs=5, space="PSUM") as psA,
            tc.tile_pool(name="psB", bufs=2, space="PSUM") as psB,
            tc.tile_pool(name="psC", bufs=1, space="PSUM") as psC,
            tc.tile_pool(name="dram", bufs=1, space="DRAM") as dram,
        ):
            nc.gpsimd.load_library(mlp)

            eidx_sb = cp.tile([128, ECOLS], I16)
            gidx_sb = cp.tile([128, GCOLS], I16)
            dstc_sb = cp.tile([128, tot_chunks], BF16)
            nrm_sb = cp.tile([128, tot_chunks], BF16)
            dv2_sb = cp.tile([128, TPC], BF16)
            w1t_sb = cp.tile([128, D], BF16)
            w2t_sb = cp.tile([128, D], BF16)
            bias_sb = cp.tile([128, 2], F32)
            iota_sb = cp.tile([128, 128], BF16)
            identb_sb = cp.tile([128, 128], BF16)
            z0_sb = cp.tile([128, TPC, D], BF16)
            z1_sb = cp.tile([128, TPC, D], BF16)
            nc.sync.dma_start(eidx_sb[:], eidx_d[:])
            nc.sync.dma_start(gidx_sb[:], gidx_d[:])
            nc.sync.dma_start(dstc_sb[:], dstc_d[:])
            nc.sync.dma_start(nrm_sb[:], nrm_d[:])
            nc.sync.dma_start(dv2_sb[:], dv2_d[:])
            nc.sync.dma_start(w1t_sb[:], w1t_d[:])
            nc.sync.dma_start(w2t_sb[:], w2t_d[:])
            nc.sync.dma_start(bias_sb[:], bias_d[:])
            nc.sync.dma_start(iota_sb[:], iota_d[:])
            nc.sync.dma_start(identb_sb[:], identb_d[:])

            warm_in = dram.tile([128], F32)
            warm_out = dram.tile([128 * NCORES], F32, addr_space="Shared")

            z0A = dram.tile([AR, D], BF16)
            z0B = dram.tile([BR, D], BF16)
            z1A = dram.tile([AR, D], BF16)
            z1B = dram.tile([BR, D], BF16)
            z0_fullA = dram.tile([NA, D], BF16, addr_space="Shared")
            z0_fullB = dram.tile([NB, D], BF16, addr_space="Shared")
            z1_fullA = dram.tile([NA, D], BF16, addr_space="Shared")
            z1_fullB = dram.tile([NB, D], BF16, addr_space="Shared")

            qn = [0]

            def next_q():
                qn[0] = (qn[0] + 1) % NQ
                return qn[0]

            def allgather(loc, full):
                nc.gpsimd.collective_compute(
                    "AllGather", mybir.AluOpType.bypass,
                    replica_groups=[list(range(NCORES))],
                    ins=[loc.opt()], outs=[full.opt()])

            def flushA(z_sb, locA, fullA):
                nc.sync.dma_start(
                    locA[:].rearrange("(t p) f -> p t f", p=128),
                    z_sb[:, 0:ATILES, :])
                if STAGE >= 2:
                    allgather(locA, fullA)

            def flushB(z_sb, locB, fullB):
                nc.sync.dma_start(
                    locB[:].rearrange("(t p) f -> p t f", p=128),
                    z_sb[:, ATILES:TPC, :])
                if STAGE >= 2:
                    allgather(locB, fullB)

            # warm up the collective path so the first real AllGather doesn't
            # pay one-time CC setup on the critical path
            if STAGE >= 2:
                allgather(warm_in, warm_out)

            # ---- embedding stage: z0 = table2[tok] (table2 = E@Wn^T+bn) ----
            for g in range(ENG):
                nidx = EGT * 128
                e_lo = embg.tile([128, EGT, D], BF16, name="e_lo", tag="e_lo")
                e_hi = embg.tile([128, EGT, D], BF16, name="e_hi", tag="e_hi")
                off = g * 2 * EGT * 8
                nc.gpsimd.dma_gather(e_lo[:], tab_lo[:],
                                     eidx_sb[:, off : off + EGT * 8],
                                     nidx, nidx, D, queue_num=next_q())
                nc.gpsimd.dma_gather(e_hi[:], tab_hi[:],
                                     eidx_sb[:, off + EGT * 8 : off + 2 * EGT * 8],
                                     nidx, nidx, D, queue_num=next_q())
                nc.vector.tensor_tensor(z0_sb[:, g * EGT : (g + 1) * EGT, :],
                                        e_lo[:], e_hi[:], ALU.add)
                if g == 2:      # tiles 0..20 done; A block = tiles 0..20
                    flushA(z0_sb, z0A, z0_fullA)
            flushB(z0_sb, z0B, z0_fullB)

            # ---- GCN layers ----
            def gcn_layer(fullA, fullB, zin_sb, wt_sb, bias_col, relu,
                          zout_sb, on_tile_done, dest, dest_f32):
                views = (fullA, fullB)
                groups = _groups()
                open_ps = {}        # group -> psum tile [128, GRP, 128]
                rself_t = {}        # group -> batched self-loop R

                def open_tile(t):
                    g = t // GRP
                    first_in_bank = g not in open_ps
                    if first_in_bank:
                        open_ps[g] = psA.tile([128, GRP, 128], F32,
                                              name="agg", tag="pA")
                        nt = len(groups[g])
                        rs = rsp.tile([128, GRP, 128], BF16, name="rs",
                                      tag="rs")
                        nc.vector.tensor_tensor(
                            rs[:, 0:nt, :],
                            identb_sb[:].unsqueeze(1)
                            .broadcast_to([128, nt, 128]),
                            dv2_sb[:, g * GRP : g * GRP + nt].unsqueeze(2)
                            .broadcast_to([128, nt, 128]),
                            ALU.mult)
                        rself_t[g] = rs
                    # start=True zeroes the whole 2KB zero region (one bank =
                    # one group of GRP tiles), so only the group's first
                    # matmul starts; siblings accumulate onto the zeroed bank.
                    nc.tensor.matmul(open_ps[g][:, t % GRP, :],
                                     zin_sb[:, t, :],
                                     rself_t[g][:, t % GRP, :],
                                     start=first_in_bank, stop=False)

                def close_group(g):
                    # runs after the bank's stop matmul: one bank-wide psum
                    # read (ordered after every matmul in the group), then
                    # per-tile projection.
                    tiles = groups[g]
                    ntile = len(tiles)
                    ps = open_ps.pop(g)
                    aggT4 = wk.tile([128, GRP, 128], BF16, name="aggT4",
                                    tag="aggT4")
                    nc.scalar.activation(aggT4[:, 0:ntile, :],
                                         ps[:, 0:ntile, :], ACT.Copy)
                    stg_t = None
                    if dest is not None:
                        stg_t = stg.tile([128, GRP, D],
                                         F32 if dest_f32 else BF16,
                                         name="stage1", tag="st1")
                    for i, t in enumerate(tiles):
                        yT_ps = psB.tile([128, 128], F32, name="yT", tag="pB")
                        nc.tensor.matmul(yT_ps[:], wt_sb[:], aggT4[:, i, :],
                                         start=True, stop=True)
                        yT_sb = wk.tile([128, 128], BF16, name="yT_sb",
                                        tag="yT_sb")
                        nc.scalar.activation(yT_sb[:], yT_ps[:],
                                             ACT.Relu if relu else ACT.Identity,
                                             bias=bias_col)
                        y_ps = psC.tile([128, 128], BF16, name="y", tag="pC")
                        nc.tensor.matmul(y_ps[:], yT_sb[:], identb_sb[:],
                                         is_transpose=True, start=True,
                                         stop=True)
                        if zout_sb is not None:
                            nc.scalar.activation(zout_sb[:, t, :], y_ps[:],
                                                 ACT.Copy)
                        else:
                            nc.scalar.activation(stg_t[:, i, :], y_ps[:],
                                                 ACT.Copy)
                    if dest is not None:
                        g0 = tiles[0]
                        dst_rows = dest[g0 * 128 : (g0 + ntile) * 128, :]
                        nc.sync.dma_start(
                            dst_rows.rearrange("(t p) f -> p t f", p=128),
                            stg_t[:, 0:ntile, :])
                    if on_tile_done is not None:
                        for t in tiles:
                            on_tile_done(t)

                for off, n, v in gathers:
                    msgs = msgp.tile([128, GMAXC, D], BF16, name="m", tag="m")
                    nc.gpsimd.dma_gather(
                        msgs[:, 0:n, :], views[v][:],
                        gidx_sb[:, off * 8 : (off + n) * 8],
                        n * 128, n * 128, D, queue_num=next_q())
                    # batched one-hot R for the gather's n chunks: two DVE
                    # ops instead of n tensor_scalars.
                    rb = rmp.tile([128, GMAXC, 128], BF16, name="r", tag="r")
                    nc.vector.tensor_tensor(
                        rb[:, 0:n, :],
                        iota_sb[:].unsqueeze(1).broadcast_to([128, n, 128]),
                        dstc_sb[:, off : off + n].unsqueeze(2)
                        .broadcast_to([128, n, 128]),
                        ALU.is_equal)
                    nc.vector.tensor_tensor(
                        rb[:, 0:n, :], rb[:, 0:n, :],
                        nrm_sb[:, off : off + n].unsqueeze(2)
                        .broadcast_to([128, n, 128]),
                        ALU.mult)
                    for k in range(n):
                        t, first, last = sched[off + k]
                        if first:
                            open_tile(t)
                        bank_last = last and (t % GRP == GRP - 1
                                              or t == TPC - 1)
                        nc.tensor.matmul(open_ps[t // GRP][:, t % GRP, :],
                                         msgs[:, k, :], rb[:, k, :],
                                         start=False, stop=bank_last)
                        if bank_last:
                            close_group(t // GRP)

            # first tile whose group-close covers all A tiles
            A_DONE_T = ((ATILES + GRP - 1) // GRP) * GRP - 1

            def l1_tile_done(t):
                if t == A_DONE_T:
                    flushA(z1_sb, z1A, z1_fullA)
                elif t == TPC - 1:
                    flushB(z1_sb, z1B, z1_fullB)

            if STAGE >= 3:
                gcn_layer(z0_fullA, z0_fullB, z0_sb, w1t_sb, bias_sb[:, 0:1],
                          True, z1_sb, l1_tile_done, None, False)
            if STAGE >= 4:
                gcn_layer(z1_fullA, z1_fullB, z1_sb, w2t_sb, bias_sb[:, 1:2],
                          False, None, None, out_d.ap(), True)

    nc.compile()
    return nc


_CACHE = {}


def _run(inputs, trace=False):
    import ml_dtypes

    node_tokens = np.asarray(inputs["node_tokens"])
    edge_index = np.asarray(inputs["edge_index"])
    embed_table = np.asarray(inputs["embed_table"], dtype=np.float32)
    Wn = np.asarray(inputs["W_node_w"], dtype=np.float32)
    bn = np.asarray(inputs["W_node_b"], dtype=np.float32)
    w1 = np.asarray(inputs["w1"], dtype=np.float32)
    b1 = np.asarray(inputs["b1"], dtype=np.float32)
    w2 = np.asarray(inputs["w2"], dtype=np.float32)
    b2 = np.asarray(inputs["b2"], dtype=np.float32)

    per_core, layout = _preprocess(node_tokens, edge_index)

    if "nc" not in _CACHE:
        _CACHE["nc"] = _build(layout)
    nc = _CACHE["nc"]

    table2 = embed_table @ Wn.T + bn            # [V, 128] f32
    tab_lo = np.concatenate([table2[:VLO], np.zeros((1, D), np.float32)]
                            ).astype(ml_dtypes.bfloat16)
    tab_hi = np.concatenate([table2[VLO:], np.zeros((1, D), np.float32)]
                            ).astype(ml_dtypes.bfloat16)
    bias = np.stack([b1, b2], axis=1).astype(np.float32)
    iota = np.tile(np.arange(128, dtype=np.float32), (128, 1)
                   ).astype(ml_dtypes.bfloat16)
    identb = np.eye(128, dtype=ml_dtypes.bfloat16)

    in_maps = []
    for c in range(NCORES):
        in_maps.append({
            "tab_lo": tab_lo, "tab_hi": tab_hi,
            "eidx": per_core[c]["eidx"],
            "gidx": per_core[c]["gidx"],
            "dstc": per_core[c]["dstc"].astype(ml_dtypes.bfloat16),
            "nrm": per_core[c]["nrm"].astype(ml_dtypes.bfloat16),
            "dv2": per_core[c]["dv2"].astype(ml_dtypes.bfloat16),
            "w1t": w1.T.astype(ml_dtypes.bfloat16),
            "w2t": w2.T.astype(ml_dtypes.bfloat16),
            "bias": bias, "iota": iota, "identb": identb,
        })

    res = run_bass_kernel_spmd(nc, in_maps, core_ids=list(range(NCORES)),
                               trace=trace)
    out = np.concatenate([res.results[c]["out"][:NPC] for c in range(NCORES)],
                         axis=0)
    return out.astype(np.float32), res


def kernel(**inputs):
    out, _ = _run(inputs, trace=False)
    return out
